# revision 112
# baseline (speedup 1.0000x reference)
"""GQA attention kernel for Trainium2, 8-core tensor-parallel.

Sharding: core c handles batch b=c//4 and kv-head pair {2*(c%4), 2*(c%4)+1}
(8 q heads). q/k/v projections column-sharded, out_proj row-sharded; the
4 partial out_proj products per batch are summed on host (the gather).

Everything on-device is feature-major ([feat, token]) so every matmul
contraction dim lands on partitions. float32r (tf32-like, full PE rate for
free-dim>=256) is used for all matmuls. Softmax has no max-subtraction
(scores are O(1) here) so exp needs no rescaling pass; the softmax
denominator comes free from a ones-column appended to V.

Schedule notes (v2): causal mask applied ON PE via a tiny bf16 matmul
(triT stationary x identity moving adds -1e32 to the upper triangle of the
diagonal 128x128 block) so the scores->exp->PV chain never leaves PE+ACT.
Attention works on lq chunks of 512 with k-tile PAIRS sharing one
[128,1024] PSUM tile so each exp instruction covers ~2 tiles (amortizes
ACT init overhead). Weights DMA is ordered kw/vw/x-first so PE starts
within ~7us. out_proj runs on [128,1024] token units with direct
PSUM->SBUF->DRAM pipeline; ow is prefetched during attention.
"""
import sys
if "/opt/trn_rl_repo" not in sys.path:
    sys.path.insert(0, "/opt/trn_rl_repo")
import numpy as np

HID = 2048
L = 2048
D = 64
NCORE = 8
NKT = HID // 128        # 16 k-tiles over hidden
NCH = 4                 # token chunks of 512 for projections
CH = 512
NLT = L // 128          # 16 lk tiles
CC = 512                # lq chunk for attention
NCC = L // CC           # 4
BIG = -1e32

_cached = {}


def _build():
    import concourse.bass as bass
    from concourse import bacc
    import concourse.mybir as mybir
    import concourse.tile as tile

    F32R = mybir.dt.float32r
    F32 = mybir.dt.float32
    BF16 = mybir.dt.bfloat16
    EXP = mybir.ActivationFunctionType.Exp

    nc = bacc.Bacc(None, target_bir_lowering=False)
    xT = nc.dram_tensor("xT", [128, NKT, L], F32R, kind="ExternalInput")
    qw = nc.dram_tensor("qw", [128, NKT, 512], F32R, kind="ExternalInput")
    kw = nc.dram_tensor("kw", [128, NKT, 128], F32R, kind="ExternalInput")
    vw = nc.dram_tensor("vw", [128, NKT, 128], F32R, kind="ExternalInput")
    ow = nc.dram_tensor("ow", [128, 4, HID], F32R, kind="ExternalInput")
    qb = nc.dram_tensor("qb", [128, 4], F32, kind="ExternalInput")
    kb = nc.dram_tensor("kb", [128, 1], F32, kind="ExternalInput")
    vb = nc.dram_tensor("vb", [128, 1], F32, kind="ExternalInput")
    triT = nc.dram_tensor("triT", [128, 128], BF16, kind="ExternalInput")
    allm = nc.dram_tensor("allm", [128, 128], BF16, kind="ExternalInput")
    idnb = nc.dram_tensor("idnb", [128, 128], BF16, kind="ExternalInput")
    ident = nc.dram_tensor("ident", [128, 128], F32, kind="ExternalInput")
    outp = nc.dram_tensor("outp", [NKT, 128, L], F32, kind="ExternalOutput")

    with tile.TileContext(nc) as tc:
        with tc.tile_pool(name="cst", bufs=1) as cst, \
             tc.tile_pool(name="res", bufs=1) as res:
            qb_sb = cst.tile([128, 4], F32)
            kb_sb = cst.tile([128, 1], F32)
            vb_sb = cst.tile([128, 1], F32)
            triT_sb = cst.tile([128, 128], BF16)
            allm_sb = cst.tile([128, 128], BF16)
            idnb_sb = cst.tile([128, 128], BF16)
            idn = cst.tile([128, 128], F32)
            def load_consts():
                # emitted after the first weight/x wave: tiny transfers,
                # none needed before ~10us in
                for dst, src in [(qb_sb, qb), (kb_sb, kb), (vb_sb, vb),
                                 (triT_sb, triT), (allm_sb, allm),
                                 (idnb_sb, idnb), (idn, ident)]:
                    nc.sync.dma_start(out=dst, in_=src.ap())

            qT_sb = res.tile([128, 4, L], F32R)   # head h: parts 64*(h//4), tile h%4
            kT_sb = res.tile([128, L], F32R)      # kv j at parts 64j
            v_aug = res.tile([128, NLT, 130], F32R)
            yT_cs = [res.tile([128, 4, CC], F32R, tag=f"yT{c}", name=f"yT{c}")
                     for c in range(NCC)]
            nc.vector.memset(v_aug[:, :, 64:65].bitcast(F32), 1.0)
            nc.vector.memset(v_aug[:, :, 129:130].bitcast(F32), 1.0)

            # ---- Phase A: projections (feature-major) + inline V transpose ----
            with tc.tile_pool(name="wqkv", bufs=1) as wpool, \
                 tc.tile_pool(name="xc", bufs=2) as xcp, \
                 tc.tile_pool(name="vtmp", bufs=2) as vtp, \
                 tc.tile_pool(name="pps", bufs=1, space="PSUM") as pps, \
                 tc.tile_pool(name="trps", bufs=2, space="PSUM") as trps:
                kw_sb = wpool.tile([128, NKT, 128], F32R)
                vw_sb = wpool.tile([128, NKT, 128], F32R)
                qw_sb = wpool.tile([128, NKT, 512], F32R)

                def load_half(hh, lo=0, hi=8):
                    nt, half = hh // 2, hh % 2
                    ts = []
                    for i in range(lo, hi):
                        t = xcp.tile([128, CH], F32R, tag=f"xc{i}")
                        kt = 8 * half + i
                        nc.sync.dma_start(
                            out=t, in_=xT.ap()[:, kt, CH * nt:CH * nt + CH])
                        ts.append(t)
                    return ts

                def wslice(w, wsb, half, mt=None, q=None):
                    ks = (slice(8 * half, 8 * half + 8) if q is None
                          else slice(8 * half + 4 * q, 8 * half + 4 * q + 4))
                    if mt is None:
                        nc.sync.dma_start(out=wsb[:, ks, :], in_=w.ap()[:, ks, :])
                    else:
                        cs = slice(128 * mt, 128 * mt + 128)
                        nc.sync.dma_start(out=wsb[:, ks, cs], in_=w.ap()[:, ks, cs])

                # transfer order tuned so the PE can start at ~5us and stays
                # fed through chunk 0 (DMA bus is the startup bottleneck)
                # PE p-state warmup: the tensor engine needs ~3us of
                # continuous work to reach 2.4GHz; real matmuls only arrive
                # once weights land (~4.5us). Chew on a zeroed scratch tile
                # meanwhile so the first real matmuls run at full clock.
                warm_sb = xcp.tile([128, CH], F32R, tag="warm")
                nc.vector.memset(warm_sb.bitcast(F32), 0.0)
                for wi in range(7):
                    warm_ps = trps.tile([128, CH], F32, tag="tr",
                                        name=f"warm{wi}")
                    nc.tensor.matmul(warm_ps, warm_sb[:, 0:128], warm_sb,
                                     start=True, stop=True)
                wslice(kw, kw_sb, 0, q=0)
                xc_next = load_half(0, 0, 1)
                wslice(vw, vw_sb, 0, q=0)
                xc_next += load_half(0, 1, 2)
                wslice(kw, kw_sb, 0, q=1)
                wslice(vw, vw_sb, 0, q=1)
                xc_next += load_half(0, 2, 4)
                wslice(qw, qw_sb, 0, 0)
                wslice(qw, qw_sb, 0, 1)
                xc_next += load_half(0, 4, 8)
                wslice(qw, qw_sb, 0, 2)
                wslice(qw, qw_sb, 0, 3)
                wslice(kw, kw_sb, 1)
                wslice(vw, vw_sb, 1)
                xc_h1 = load_half(1)
                for mt in range(4):
                    wslice(qw, qw_sb, 1, mt)
                load_consts()

                for nt in range(NCH):
                    sl = slice(CH * nt, CH * nt + CH)
                    xcs = [None, None]
                    for half in range(2):
                        if nt == 0:
                            xcs[half] = xc_next if half == 0 else xc_h1
                        else:
                            xcs[half] = load_half(2 * nt + half)
                    q_ps = [pps.tile([128, CH], F32, tag=f"qps{m}", name=f"qps{m}")
                            for m in range(4)]
                    k_ps = pps.tile([128, CH], F32, tag="kps")
                    v_ps = pps.tile([128, CH], F32, tag="vps")
                    for half in range(2):
                        for i in range(8):
                            kt = 8 * half + i
                            st, sp = kt == 0, kt == NKT - 1
                            nc.tensor.matmul(k_ps, kw_sb[:, kt, :], xcs[half][i],
                                             start=st, stop=sp)
                            nc.tensor.matmul(v_ps, vw_sb[:, kt, :], xcs[half][i],
                                             start=st, stop=sp)
                            for mt in range(4):
                                nc.tensor.matmul(
                                    q_ps[mt], qw_sb[:, kt, 128 * mt:128 * mt + 128],
                                    xcs[half][i], start=st, stop=sp)
                    vtmp = vtp.tile([128, CH], F32, tag="vt")
                    nc.scalar.add(vtmp, v_ps, vb_sb)
                    for tt in range(CH // 128):
                        t = (CH * nt) // 128 + tt
                        tr_ps = trps.tile([128, 128], F32, tag="tr")
                        nc.tensor.transpose(tr_ps, vtmp[:, 128 * tt:128 * tt + 128], idn)
                        nc.vector.tensor_copy(out=v_aug[:, t, 0:64], in_=tr_ps[:, 0:64])
                        nc.vector.tensor_copy(out=v_aug[:, t, 65:129], in_=tr_ps[:, 64:128])
                    nc.scalar.add(kT_sb[:, sl], k_ps, kb_sb)
                    for mt in range(4):
                        if mt % 2 == 1:
                            nc.scalar.add(qT_sb[:, mt, sl], q_ps[mt],
                                          qb_sb[:, mt:mt + 1])
                        else:
                            nc.vector.tensor_scalar_add(
                                out=qT_sb[:, mt, sl], in0=q_ps[mt],
                                scalar1=qb_sb[:, mt:mt + 1])

            # ---- Phase C: attention, scores transposed [lk, lq] ----
            # lq chunks of 512; k-tiles processed in pairs sharing one
            # [128,1024] PSUM tile so each exp covers ~2 tiles.
            ow_sb = res.tile([128, 4, HID], F32R)
            with tc.tile_pool(name="work", bufs=8) as work, \
                 tc.tile_pool(name="od", bufs=6) as od, \
                 tc.tile_pool(name="scps", bufs=2, space="PSUM") as scps, \
                 tc.tile_pool(name="pvps", bufs=2, space="PSUM") as pvps, \
                 tc.tile_pool(name="ops", bufs=2, space="PSUM") as ops:
                nc.sync.dma_start(out=ow_sb, in_=ow.ap())
                for cc in range(NCC):
                    # short chunks (few pairs) pipeline poorly within one
                    # head; interleave two heads' chains to hide the
                    # scores->exp->PV latency. Longer chunks run head-at-a-
                    # time (deeper in-head pipeline, pv slots rotate).
                    G = 2 if cc < 2 else 1
                    # cc3 ends with h=3: its epilogue writes yT directly on
                    # DVE (h>=4 heads need an extra SBUF-shift DMA, which
                    # would sit on the kernel's terminal dependency chain)
                    horder = ((0, 4, 1, 5, 2, 6, 7, 3) if cc == NCC - 1
                              else (0, 4, 1, 5, 2, 6, 3, 7))
                    npair = 2 * cc + 2
                    for g0 in range(0, 8, G):
                        heads = [horder[g0 + i] for i in range(G)]
                        pvt = {}
                        for h in heads:
                            pvt[h] = pvps.tile([65, CC], F32, tag="pv",
                                               name=f"pv{cc}_{h}")
                        exps = []
                        for pr in range(npair):
                            for h in heads:
                                base, mt = 64 * (h // 4), h % 4
                                sc = scps.tile([128, 2 * CC], F32, tag="sc")
                                po0 = 0
                                for ci, ti in enumerate((2 * pr, 2 * pr + 1)):
                                    cb = CC * ci
                                    o = max(0, 128 * ti - CC * cc)
                                    po = 256 if o == 384 else o
                                    if ci == 0:
                                        po0 = po
                                    nc.tensor.matmul(
                                        sc[:, cb + po:cb + CC],
                                        kT_sb[base:base + 64, 128 * ti:128 * ti + 128],
                                        qT_sb[base:base + 64, mt, CC * cc + po:CC * cc + CC],
                                        start=True, stop=True)
                                    if ti >= 4 * cc:
                                        if po != o:
                                            nc.tensor.matmul(
                                                sc[:, cb + po:cb + o],
                                                allm_sb, idnb_sb,
                                                start=False, stop=True,
                                                skip_group_check=True)
                                        nc.tensor.matmul(
                                            sc[:, cb + o:cb + o + 128],
                                            triT_sb, idnb_sb,
                                            start=False, stop=True,
                                            skip_group_check=True)
                                expS = work.tile([128, 2 * CC], F32R, tag="expS")
                                nc.scalar.activation(out=expS[:, po0:2 * CC],
                                                     in_=sc[:, po0:2 * CC],
                                                     func=EXP, scale=0.125)
                                exps.append((expS, pr, h))
                        for expS, pr, h in exps:
                            j = h // 4
                            for ci, ti in enumerate((2 * pr, 2 * pr + 1)):
                                cb = CC * ci
                                o = max(0, 128 * ti - CC * cc)
                                po = 256 if o == 384 else o
                                nc.tensor.matmul(
                                    pvt[h][:, po:CC],
                                    v_aug[:, ti, 65 * j:65 * j + 65],
                                    expS[:, cb + po:cb + CC],
                                    start=(pr == 0 and ci == 0),
                                    stop=(pr == npair - 1 and ci == 1))
                        for h in heads:
                            base, mt = 64 * (h // 4), h % 4
                            pv_ps = pvt[h]
                            # terminal heads (mt3, last chunk) gate the final
                            # out_proj units: pipeline their normalize chain
                            # in 256-col halves to shorten the kernel tail
                            cuts = ((0, CC) if not (cc == NCC - 1 and mt == 3)
                                    else (0, CC // 2, CC))
                            recip = work.tile([1, CC], F32, tag="recip")
                            bcast = work.tile([64, CC], F32, tag="bcast")
                            ytmp = None
                            if h >= 4:
                                ytmp = work.tile([64, CC], F32R, tag="ytmp",
                                                 name=f"ytmp{cc}_{h}")
                            for zi in range(len(cuts) - 1):
                                sp_ = slice(cuts[zi], cuts[zi + 1])
                                nc.vector.reciprocal(recip[:, sp_],
                                                     pv_ps[64:65, sp_])
                                nc.gpsimd.partition_broadcast(bcast[:, sp_],
                                                              recip[:, sp_])
                                if h < 4:
                                    nc.vector.tensor_mul(
                                        out=yT_cs[cc][0:64, mt, sp_],
                                        in0=pv_ps[0:64, sp_], in1=bcast[:, sp_])
                                else:
                                    nc.vector.tensor_mul(out=ytmp[:, sp_],
                                                         in0=pv_ps[0:64, sp_],
                                                         in1=bcast[:, sp_])
                                    nc.sync.dma_start(
                                        out=yT_cs[cc][64:128, mt, sp_],
                                        in_=ytmp[:, sp_])

                # ---- Phase D: out_proj, merged into this scope so its
                # matmuls fill PE idle gaps in the ACT-bound attention tail.
                # 512-token units: 1 PSUM bank each, bufs=2.
                for tp in range(4):
                    for ot in range(NKT):
                        # tail units (tp=3) run after attention has released
                        # the pv slots; alternate onto them (same 1-bank
                        # footprint) to deepen the mm->copy->DMA rotation
                        otag = "pv" if tp == 3 and ot % 2 == 1 else "o"
                        opool = pvps if otag == "pv" else ops
                        o_ps = opool.tile([128, CC], F32, tag=otag)
                        for it in range(4):
                            nc.tensor.matmul(
                                o_ps,
                                ow_sb[:, it, 128 * ot:128 * ot + 128],
                                yT_cs[tp][:, it, :],
                                start=(it == 0), stop=(it == 3))
                        o_sb = od.tile([128, CC], F32, tag="osb")
                        nc.vector.tensor_copy(out=o_sb, in_=o_ps)
                        if tp == 3 and ot >= 14:
                            # terminal stores issue from ACT's idle HWDGE so
                            # they don't queue behind SP's in-order issue slots
                            nc.scalar.dma_start(
                                out=outp.ap()[ot, :, CC * tp:CC * tp + CC],
                                in_=o_sb)
                        else:
                            nc.sync.dma_start(
                                out=outp.ap()[ot, :, CC * tp:CC * tp + CC],
                                in_=o_sb)
    nc.compile()
    return nc


def _perm512():
    p = np.empty(512, dtype=np.int64)
    for mt in range(4):
        for half in range(2):
            head = mt + 4 * half
            p[128 * mt + 64 * half:128 * mt + 64 * half + 64] = \
                np.arange(64 * head, 64 * head + 64)
    return p


def kernel(x, attention_mask, q_w, q_b, k_w, k_b, v_w, v_b, o_w, o_b):
    from concourse.bass_utils import run_bass_kernel_spmd
    import ml_dtypes

    x = np.asarray(x, dtype=np.float32)
    q_w = np.asarray(q_w, dtype=np.float32); q_b = np.asarray(q_b, dtype=np.float32)
    k_w = np.asarray(k_w, dtype=np.float32); k_b = np.asarray(k_b, dtype=np.float32)
    v_w = np.asarray(v_w, dtype=np.float32); v_b = np.asarray(v_b, dtype=np.float32)
    o_w = np.asarray(o_w, dtype=np.float32); o_b = np.asarray(o_b, dtype=np.float32)
    am = np.asarray(attention_mask)
    assert am.all(), "kernel assumes attention_mask == all ones"

    if "nc" not in _cached:
        _cached["nc"] = _build()
    nc = _cached["nc"]

    perm = _perm512()
    tri_np = np.where(np.arange(128)[:, None] > np.arange(128)[None, :],
                      np.float32(BIG), np.float32(0)).astype(np.float32)
    triT_np = np.ascontiguousarray(tri_np.T).astype(ml_dtypes.bfloat16)
    allm_np = np.full((128, 128), np.float32(BIG)).astype(ml_dtypes.bfloat16)
    idnb_np = np.eye(128, dtype=np.float32).astype(ml_dtypes.bfloat16)
    id_np = np.eye(128, dtype=np.float32)

    in_maps = []
    for c in range(NCORE):
        b, g = c // 4, c % 4
        G0 = 512 * g
        xT_t = np.ascontiguousarray(
            x[b].T.reshape(NKT, 128, L).transpose(1, 0, 2))
        qws = q_w[G0:G0 + 512][perm]
        qw_t = np.ascontiguousarray(qws.T.reshape(NKT, 128, 512).transpose(1, 0, 2))
        kws = k_w[128 * g:128 * g + 128]
        kw_t = np.ascontiguousarray(kws.T.reshape(NKT, 128, 128).transpose(1, 0, 2))
        vws = v_w[128 * g:128 * g + 128]
        vw_t = np.ascontiguousarray(vws.T.reshape(NKT, 128, 128).transpose(1, 0, 2))
        owp = o_w[:, G0:G0 + 512][:, perm]
        ow_t = np.ascontiguousarray(owp.T.reshape(4, 128, HID).transpose(1, 0, 2))
        qb_t = np.ascontiguousarray(q_b[G0:G0 + 512][perm].reshape(4, 128).T)
        kb_t = k_b[128 * g:128 * g + 128].reshape(128, 1).copy()
        vb_t = v_b[128 * g:128 * g + 128].reshape(128, 1).copy()
        in_maps.append({"xT": xT_t, "qw": qw_t, "kw": kw_t, "vw": vw_t,
                        "ow": ow_t, "qb": qb_t, "kb": kb_t, "vb": vb_t,
                        "triT": triT_np, "allm": allm_np, "idnb": idnb_np,
                        "ident": id_np})

    res = run_bass_kernel_spmd(nc, in_maps, core_ids=list(range(NCORE)))
    out = np.empty((2, L, HID), dtype=np.float32)
    for b in range(2):
        acc = res.results[4 * b]["outp"].astype(np.float32).copy()
        for i in range(1, 4):
            acc += res.results[4 * b + i]["outp"]
        out[b] = acc.reshape(HID, L).T + o_b
    return out



# revision 113
# speedup vs baseline: 1.0005x; 1.0005x over previous
"""GQA attention kernel for Trainium2, 8-core tensor-parallel.

Sharding: core c handles batch b=c//4 and kv-head pair {2*(c%4), 2*(c%4)+1}
(8 q heads). q/k/v projections column-sharded, out_proj row-sharded; the
4 partial out_proj products per batch are summed on host (the gather).

Everything on-device is feature-major ([feat, token]) so every matmul
contraction dim lands on partitions. float32r (tf32-like, full PE rate for
free-dim>=256) is used for all matmuls. Softmax has no max-subtraction
(scores are O(1) here) so exp needs no rescaling pass; the softmax
denominator comes free from a ones-column appended to V.

Schedule notes (v2): causal mask applied ON PE via a tiny bf16 matmul
(triT stationary x identity moving adds -1e32 to the upper triangle of the
diagonal 128x128 block) so the scores->exp->PV chain never leaves PE+ACT.
Attention works on lq chunks of 512 with k-tile PAIRS sharing one
[128,1024] PSUM tile so each exp instruction covers ~2 tiles (amortizes
ACT init overhead). Weights DMA is ordered kw/vw/x-first so PE starts
within ~7us. out_proj runs on [128,1024] token units with direct
PSUM->SBUF->DRAM pipeline; ow is prefetched during attention.
"""
import sys
if "/opt/trn_rl_repo" not in sys.path:
    sys.path.insert(0, "/opt/trn_rl_repo")
import numpy as np

HID = 2048
L = 2048
D = 64
NCORE = 8
NKT = HID // 128        # 16 k-tiles over hidden
NCH = 4                 # token chunks of 512 for projections
CH = 512
NLT = L // 128          # 16 lk tiles
CC = 512                # lq chunk for attention
NCC = L // CC           # 4
BIG = -1e32

_cached = {}


def _build():
    import concourse.bass as bass
    from concourse import bacc
    import concourse.mybir as mybir
    import concourse.tile as tile

    F32R = mybir.dt.float32r
    F32 = mybir.dt.float32
    BF16 = mybir.dt.bfloat16
    EXP = mybir.ActivationFunctionType.Exp

    nc = bacc.Bacc(None, target_bir_lowering=False)
    xT = nc.dram_tensor("xT", [128, NKT, L], F32R, kind="ExternalInput")
    qw = nc.dram_tensor("qw", [128, NKT, 512], F32R, kind="ExternalInput")
    kw = nc.dram_tensor("kw", [128, NKT, 128], F32R, kind="ExternalInput")
    vw = nc.dram_tensor("vw", [128, NKT, 128], F32R, kind="ExternalInput")
    ow = nc.dram_tensor("ow", [128, 4, HID], F32R, kind="ExternalInput")
    qb = nc.dram_tensor("qb", [128, 4], F32, kind="ExternalInput")
    kb = nc.dram_tensor("kb", [128, 1], F32, kind="ExternalInput")
    vb = nc.dram_tensor("vb", [128, 1], F32, kind="ExternalInput")
    triT = nc.dram_tensor("triT", [128, 128], BF16, kind="ExternalInput")
    allm = nc.dram_tensor("allm", [128, 128], BF16, kind="ExternalInput")
    idnb = nc.dram_tensor("idnb", [128, 128], BF16, kind="ExternalInput")
    ident = nc.dram_tensor("ident", [128, 128], F32, kind="ExternalInput")
    outp = nc.dram_tensor("outp", [NKT, 128, L], F32, kind="ExternalOutput")

    with tile.TileContext(nc) as tc:
        with tc.tile_pool(name="cst", bufs=1) as cst, \
             tc.tile_pool(name="res", bufs=1) as res:
            qb_sb = cst.tile([128, 4], F32)
            kb_sb = cst.tile([128, 1], F32)
            vb_sb = cst.tile([128, 1], F32)
            triT_sb = cst.tile([128, 128], BF16)
            allm_sb = cst.tile([128, 128], BF16)
            idnb_sb = cst.tile([128, 128], BF16)
            idn = cst.tile([128, 128], F32)
            def load_consts():
                # emitted after the first weight/x wave: tiny transfers,
                # none needed before ~10us in
                for dst, src in [(qb_sb, qb), (kb_sb, kb), (vb_sb, vb),
                                 (triT_sb, triT), (allm_sb, allm),
                                 (idnb_sb, idnb), (idn, ident)]:
                    nc.sync.dma_start(out=dst, in_=src.ap())

            qT_sb = res.tile([128, 4, L], F32R)   # head h: parts 64*(h//4), tile h%4
            kT_sb = res.tile([128, L], F32R)      # kv j at parts 64j
            v_aug = res.tile([128, NLT, 130], F32R)
            yT_cs = [res.tile([128, 4, CC], F32R, tag=f"yT{c}", name=f"yT{c}")
                     for c in range(NCC)]
            nc.vector.memset(v_aug[:, :, 64:65].bitcast(F32), 1.0)
            nc.vector.memset(v_aug[:, :, 129:130].bitcast(F32), 1.0)

            # ---- Phase A: projections (feature-major) + inline V transpose ----
            with tc.tile_pool(name="wqkv", bufs=1) as wpool, \
                 tc.tile_pool(name="xc", bufs=2) as xcp, \
                 tc.tile_pool(name="vtmp", bufs=2) as vtp, \
                 tc.tile_pool(name="pps", bufs=1, space="PSUM") as pps, \
                 tc.tile_pool(name="trps", bufs=2, space="PSUM") as trps:
                kw_sb = wpool.tile([128, NKT, 128], F32R)
                vw_sb = wpool.tile([128, NKT, 128], F32R)
                qw_sb = wpool.tile([128, NKT, 512], F32R)

                def load_half(hh, lo=0, hi=8):
                    nt, half = hh // 2, hh % 2
                    ts = []
                    for i in range(lo, hi):
                        t = xcp.tile([128, CH], F32R, tag=f"xc{i}")
                        kt = 8 * half + i
                        nc.sync.dma_start(
                            out=t, in_=xT.ap()[:, kt, CH * nt:CH * nt + CH])
                        ts.append(t)
                    return ts

                def wslice(w, wsb, half, mt=None, q=None):
                    ks = (slice(8 * half, 8 * half + 8) if q is None
                          else slice(8 * half + 4 * q, 8 * half + 4 * q + 4))
                    if mt is None:
                        nc.sync.dma_start(out=wsb[:, ks, :], in_=w.ap()[:, ks, :])
                    else:
                        cs = slice(128 * mt, 128 * mt + 128)
                        nc.sync.dma_start(out=wsb[:, ks, cs], in_=w.ap()[:, ks, cs])

                # transfer order tuned so the PE can start at ~5us and stays
                # fed through chunk 0 (DMA bus is the startup bottleneck)
                # PE p-state warmup: the tensor engine needs ~3us of
                # continuous work to reach 2.4GHz; real matmuls only arrive
                # once weights land (~4.5us). Chew on a zeroed scratch tile
                # meanwhile so the first real matmuls run at full clock.
                warm_sb = xcp.tile([128, CH], F32R, tag="warm")
                nc.vector.memset(warm_sb.bitcast(F32), 0.0)
                for wi in range(7):
                    warm_ps = trps.tile([128, CH], F32, tag="tr",
                                        name=f"warm{wi}")
                    nc.tensor.matmul(warm_ps, warm_sb[:, 0:128], warm_sb,
                                     start=True, stop=True)
                wslice(kw, kw_sb, 0, q=0)
                xc_next = load_half(0, 0, 1)
                wslice(vw, vw_sb, 0, q=0)
                xc_next += load_half(0, 1, 2)
                wslice(kw, kw_sb, 0, q=1)
                wslice(vw, vw_sb, 0, q=1)
                xc_next += load_half(0, 2, 4)
                wslice(qw, qw_sb, 0, 0)
                wslice(qw, qw_sb, 0, 1)
                xc_next += load_half(0, 4, 8)
                wslice(qw, qw_sb, 0, 2)
                wslice(qw, qw_sb, 0, 3)
                wslice(kw, kw_sb, 1)
                wslice(vw, vw_sb, 1)
                xc_h1 = load_half(1)
                for mt in range(4):
                    wslice(qw, qw_sb, 1, mt)
                load_consts()

                for nt in range(NCH):
                    sl = slice(CH * nt, CH * nt + CH)
                    xcs = [None, None]
                    for half in range(2):
                        if nt == 0:
                            xcs[half] = xc_next if half == 0 else xc_h1
                        else:
                            xcs[half] = load_half(2 * nt + half)
                    q_ps = [pps.tile([128, CH], F32, tag=f"qps{m}", name=f"qps{m}")
                            for m in range(4)]
                    k_ps = pps.tile([128, CH], F32, tag="kps")
                    v_ps = pps.tile([128, CH], F32, tag="vps")
                    for half in range(2):
                        for i in range(8):
                            kt = 8 * half + i
                            st, sp = kt == 0, kt == NKT - 1
                            nc.tensor.matmul(k_ps, kw_sb[:, kt, :], xcs[half][i],
                                             start=st, stop=sp)
                            nc.tensor.matmul(v_ps, vw_sb[:, kt, :], xcs[half][i],
                                             start=st, stop=sp)
                            for mt in range(4):
                                nc.tensor.matmul(
                                    q_ps[mt], qw_sb[:, kt, 128 * mt:128 * mt + 128],
                                    xcs[half][i], start=st, stop=sp)
                    vtmp = vtp.tile([128, CH], F32, tag="vt")
                    nc.scalar.add(vtmp, v_ps, vb_sb)
                    for tt in range(CH // 128):
                        t = (CH * nt) // 128 + tt
                        tr_ps = trps.tile([128, 128], F32, tag="tr")
                        nc.tensor.transpose(tr_ps, vtmp[:, 128 * tt:128 * tt + 128], idn)
                        nc.vector.tensor_copy(out=v_aug[:, t, 0:64], in_=tr_ps[:, 0:64])
                        nc.vector.tensor_copy(out=v_aug[:, t, 65:129], in_=tr_ps[:, 64:128])
                    nc.scalar.add(kT_sb[:, sl], k_ps, kb_sb)
                    for mt in range(4):
                        if mt % 2 == 1:
                            nc.scalar.add(qT_sb[:, mt, sl], q_ps[mt],
                                          qb_sb[:, mt:mt + 1])
                        else:
                            nc.vector.tensor_scalar_add(
                                out=qT_sb[:, mt, sl], in0=q_ps[mt],
                                scalar1=qb_sb[:, mt:mt + 1])

            # ---- Phase C: attention, scores transposed [lk, lq] ----
            # lq chunks of 512; k-tiles processed in pairs sharing one
            # [128,1024] PSUM tile so each exp covers ~2 tiles.
            ow_sb = res.tile([128, 4, HID], F32R)
            with tc.tile_pool(name="work", bufs=8) as work, \
                 tc.tile_pool(name="od", bufs=6) as od, \
                 tc.tile_pool(name="scps", bufs=2, space="PSUM") as scps, \
                 tc.tile_pool(name="pvps", bufs=2, space="PSUM") as pvps, \
                 tc.tile_pool(name="ops", bufs=2, space="PSUM") as ops:
                nc.sync.dma_start(out=ow_sb, in_=ow.ap())
                for cc in range(NCC):
                    # short chunks (few pairs) pipeline poorly within one
                    # head; interleave two heads' chains to hide the
                    # scores->exp->PV latency. Longer chunks run head-at-a-
                    # time (deeper in-head pipeline, pv slots rotate).
                    G = 2 if cc < 2 else 1
                    # cc3 ends with h=3: its epilogue writes yT directly on
                    # DVE (h>=4 heads need an extra SBUF-shift DMA, which
                    # would sit on the kernel's terminal dependency chain)
                    horder = ((0, 4, 1, 5, 2, 6, 7, 3) if cc == NCC - 1
                              else (0, 4, 1, 5, 2, 6, 3, 7))
                    npair = 2 * cc + 2
                    for g0 in range(0, 8, G):
                        heads = [horder[g0 + i] for i in range(G)]
                        pvt = {}
                        for h in heads:
                            pvt[h] = pvps.tile([65, CC], F32, tag="pv",
                                               name=f"pv{cc}_{h}")
                        exps = []
                        for pr in range(npair):
                            for h in heads:
                                base, mt = 64 * (h // 4), h % 4
                                sc = scps.tile([128, 2 * CC], F32, tag="sc")
                                po0 = 0
                                for ci, ti in enumerate((2 * pr, 2 * pr + 1)):
                                    cb = CC * ci
                                    o = max(0, 128 * ti - CC * cc)
                                    po = 256 if o == 384 else o
                                    if ci == 0:
                                        po0 = po
                                    nc.tensor.matmul(
                                        sc[:, cb + po:cb + CC],
                                        kT_sb[base:base + 64, 128 * ti:128 * ti + 128],
                                        qT_sb[base:base + 64, mt, CC * cc + po:CC * cc + CC],
                                        start=True, stop=True)
                                    if ti >= 4 * cc:
                                        if po != o:
                                            nc.tensor.matmul(
                                                sc[:, cb + po:cb + o],
                                                allm_sb, idnb_sb,
                                                start=False, stop=True,
                                                skip_group_check=True)
                                        nc.tensor.matmul(
                                            sc[:, cb + o:cb + o + 128],
                                            triT_sb, idnb_sb,
                                            start=False, stop=True,
                                            skip_group_check=True)
                                expS = work.tile([128, 2 * CC], F32R, tag="expS")
                                nc.scalar.activation(out=expS[:, po0:2 * CC],
                                                     in_=sc[:, po0:2 * CC],
                                                     func=EXP, scale=0.125)
                                exps.append((expS, pr, h))
                        for expS, pr, h in exps:
                            j = h // 4
                            for ci, ti in enumerate((2 * pr, 2 * pr + 1)):
                                cb = CC * ci
                                o = max(0, 128 * ti - CC * cc)
                                po = 256 if o == 384 else o
                                nc.tensor.matmul(
                                    pvt[h][:, po:CC],
                                    v_aug[:, ti, 65 * j:65 * j + 65],
                                    expS[:, cb + po:cb + CC],
                                    start=(pr == 0 and ci == 0),
                                    stop=(pr == npair - 1 and ci == 1))
                        for h in heads:
                            base, mt = 64 * (h // 4), h % 4
                            pv_ps = pvt[h]
                            # terminal heads (mt3, last chunk) gate the final
                            # out_proj units: pipeline their normalize chain
                            # in 256-col halves to shorten the kernel tail
                            cuts = ((0, CC) if not (cc == NCC - 1 and mt == 3)
                                    else (0, CC // 2, CC))
                            recip = work.tile([1, CC], F32, tag="recip")
                            bcast = work.tile([64, CC], F32, tag="bcast")
                            ytmp = None
                            if h >= 4:
                                ytmp = work.tile([64, CC], F32R, tag="ytmp",
                                                 name=f"ytmp{cc}_{h}")
                            for zi in range(len(cuts) - 1):
                                sp_ = slice(cuts[zi], cuts[zi + 1])
                                nc.vector.reciprocal(recip[:, sp_],
                                                     pv_ps[64:65, sp_])
                                nc.gpsimd.partition_broadcast(bcast[:, sp_],
                                                              recip[:, sp_])
                                if h < 4:
                                    nc.vector.tensor_mul(
                                        out=yT_cs[cc][0:64, mt, sp_],
                                        in0=pv_ps[0:64, sp_], in1=bcast[:, sp_])
                                else:
                                    nc.vector.tensor_mul(out=ytmp[:, sp_],
                                                         in0=pv_ps[0:64, sp_],
                                                         in1=bcast[:, sp_])
                                    nc.sync.dma_start(
                                        out=yT_cs[cc][64:128, mt, sp_],
                                        in_=ytmp[:, sp_])

                # ---- Phase D: out_proj, merged into this scope so its
                # matmuls fill PE idle gaps in the ACT-bound attention tail.
                # 512-token units: 1 PSUM bank each, bufs=2.
                for tp in range(4):
                    for ot in range(NKT):
                        # tail units (tp=3) run after attention has released
                        # the pv slots; alternate onto them (same 1-bank
                        # footprint) to deepen the mm->copy->DMA rotation
                        otag = "pv" if tp == 3 and ot % 2 == 1 else "o"
                        opool = pvps if otag == "pv" else ops
                        o_ps = opool.tile([128, CC], F32, tag=otag)
                        for it in range(4):
                            nc.tensor.matmul(
                                o_ps,
                                ow_sb[:, it, 128 * ot:128 * ot + 128],
                                yT_cs[tp][:, it, :],
                                start=(it == 0), stop=(it == 3))
                        o_sb = od.tile([128, CC], F32, tag="osb")
                        nc.vector.tensor_copy(out=o_sb, in_=o_ps)
                        nc.sync.dma_start(
                            out=outp.ap()[ot, :, CC * tp:CC * tp + CC],
                            in_=o_sb)
    nc.compile()
    return nc


def _perm512():
    p = np.empty(512, dtype=np.int64)
    for mt in range(4):
        for half in range(2):
            head = mt + 4 * half
            p[128 * mt + 64 * half:128 * mt + 64 * half + 64] = \
                np.arange(64 * head, 64 * head + 64)
    return p


def kernel(x, attention_mask, q_w, q_b, k_w, k_b, v_w, v_b, o_w, o_b):
    from concourse.bass_utils import run_bass_kernel_spmd
    import ml_dtypes

    x = np.asarray(x, dtype=np.float32)
    q_w = np.asarray(q_w, dtype=np.float32); q_b = np.asarray(q_b, dtype=np.float32)
    k_w = np.asarray(k_w, dtype=np.float32); k_b = np.asarray(k_b, dtype=np.float32)
    v_w = np.asarray(v_w, dtype=np.float32); v_b = np.asarray(v_b, dtype=np.float32)
    o_w = np.asarray(o_w, dtype=np.float32); o_b = np.asarray(o_b, dtype=np.float32)
    am = np.asarray(attention_mask)
    assert am.all(), "kernel assumes attention_mask == all ones"

    if "nc" not in _cached:
        _cached["nc"] = _build()
    nc = _cached["nc"]

    perm = _perm512()
    tri_np = np.where(np.arange(128)[:, None] > np.arange(128)[None, :],
                      np.float32(BIG), np.float32(0)).astype(np.float32)
    triT_np = np.ascontiguousarray(tri_np.T).astype(ml_dtypes.bfloat16)
    allm_np = np.full((128, 128), np.float32(BIG)).astype(ml_dtypes.bfloat16)
    idnb_np = np.eye(128, dtype=np.float32).astype(ml_dtypes.bfloat16)
    id_np = np.eye(128, dtype=np.float32)

    in_maps = []
    for c in range(NCORE):
        b, g = c // 4, c % 4
        G0 = 512 * g
        xT_t = np.ascontiguousarray(
            x[b].T.reshape(NKT, 128, L).transpose(1, 0, 2))
        qws = q_w[G0:G0 + 512][perm]
        qw_t = np.ascontiguousarray(qws.T.reshape(NKT, 128, 512).transpose(1, 0, 2))
        kws = k_w[128 * g:128 * g + 128]
        kw_t = np.ascontiguousarray(kws.T.reshape(NKT, 128, 128).transpose(1, 0, 2))
        vws = v_w[128 * g:128 * g + 128]
        vw_t = np.ascontiguousarray(vws.T.reshape(NKT, 128, 128).transpose(1, 0, 2))
        owp = o_w[:, G0:G0 + 512][:, perm]
        ow_t = np.ascontiguousarray(owp.T.reshape(4, 128, HID).transpose(1, 0, 2))
        qb_t = np.ascontiguousarray(q_b[G0:G0 + 512][perm].reshape(4, 128).T)
        kb_t = k_b[128 * g:128 * g + 128].reshape(128, 1).copy()
        vb_t = v_b[128 * g:128 * g + 128].reshape(128, 1).copy()
        in_maps.append({"xT": xT_t, "qw": qw_t, "kw": kw_t, "vw": vw_t,
                        "ow": ow_t, "qb": qb_t, "kb": kb_t, "vb": vb_t,
                        "triT": triT_np, "allm": allm_np, "idnb": idnb_np,
                        "ident": id_np})

    res = run_bass_kernel_spmd(nc, in_maps, core_ids=list(range(NCORE)))
    out = np.empty((2, L, HID), dtype=np.float32)
    for b in range(2):
        acc = res.results[4 * b]["outp"].astype(np.float32).copy()
        for i in range(1, 4):
            acc += res.results[4 * b + i]["outp"]
        out[b] = acc.reshape(HID, L).T + o_b
    return out

